# revision 21
# baseline (speedup 1.0000x reference)
"""Mat2Twist Trainium2 kernel: batch of 3x3 rotation matrices -> twist vectors.

For each matrix R:  tr = trace(R); x = (tr-1)/2 = cos(theta)
  theta = arccos(x);  w = [R21-R12, R02-R20, R10-R01]  (|w| = 2 sin theta)
  out = theta/(2 sin theta) * w

arccos via the Abramowitz-Stegun-style split
  arccos(x) = pi/2 + sign(x)*(arccos(|x|) - pi/2),
  arccos(|x|) = sqrt(1-|x|) * q(|x|),  q deg-2 minimax (err ~1e-3)
so every activation (Abs, Sign, Square, Sqrt) lives in ONE ACT table set
("sqrt_and_others") -- a Square/Ln/Exp/Arctan mix would force ~2
ACT_TABLE_LOADs (1.3us each) per chunk and make the scalar engine the
pipeline straggler.  1/sin(theta) uses the bitwise-NOT trick: for s>0,
z = s*bitcast(~bits(s)) always lands in [-4.5,-4], and a single
Chebyshev-tuned factor k*(z-A0)*bitcast(~bits(s)) ~= 1/s to 1.7e-3
(output gate is 2e-2) -- two DVE ops, no iterative divide.  All
remaining constants fold into existing scale/scalar immediate slots.

Per chunk (m matrices per partition), tile X = [minu(3m)|subt(3m)|R00|R11|R22]:
  gp:  tr = R00+R11+R22                       [GpSimd TT x2]
  act: ax=Abs(.5tr-.5) sg=Sign(.5tr-.5) v=Square(.5tr-.5)   (x=(tr-1)/2)
       qq=Square(SQ*ax+B2)  sn=Sqrt(1-v)  S=Sqrt(1-ax)  [6 ACT, one set]
  dve: w  = minu-subt (in place)
       nx = bitcast(bits(sn) ^ -1)            [int ts]
       z  = sn*nx                             [TT]
       u  = (qq+KQ)*S       = arccos(|x|)
       p  = (u-pi/2)*sg     = theta - pi/2
       A  = (p+pi/2)*nx     = theta*nx
       P  = (z-A0)*A        = theta/(k*sin theta)
       out_j = (HALF_K*P)*w_j  as one STT over [P,3,m], P broadcast
  out-DMA on the ACT ring, inputs on the SP ring.

The six small intermediates live as slices of ONE scratch tile per
chunk (2 pool allocations per chunk, not 7): every allocation costs a
per-engine event-semaphore round that the idle engines replay in a
multi-microsecond lockstep walk at kernel teardown.

Software-pipelined emission, skew: dma(i)@i -> trace@i+1 -> acts@i+2 ->
dve@i+3 -> out-dma trigger@i+4 (trigger emitted at the top of its
iteration so it never queues behind that iteration's ACT block).  Tail
chunks shrink (256,128,128) to cut pipeline-drain latency after the
last input byte lands.
"""

import numpy as np

import concourse.bass as bass
import concourse.mybir as mybir
from concourse.tile import TileContext
from concourse.bass_utils import run_bass_kernel_spmd

B = 4194304
NCORES = 8
P = 128
N_C = B // NCORES        # 524288 matrices per core
MPP = N_C // P           # 4096 matrices per partition
MS = [512] * 7 + [256] + [128] * 2   # per-chunk matrices per partition
assert sum(MS) == MPP

# component order in DRAM (flat 3x3 index): minuends, subtrahends, diagonal
PERM = [7, 2, 3, 5, 6, 1, 0, 4, 8]

F32 = mybir.dt.float32
I32 = mybir.dt.int32
ACT = mybir.ActivationFunctionType
ALU = mybir.AluOpType
PI_2 = float(np.pi / 2.0)
MAXM = max(MS)

# deg-2 minimax fit of q(t) = arccos(t)/sqrt(1-t) on [0, cos(0.1)],
# written as q = (SQ*t + B2)^2 + KQ so one ACT Square evaluates it.
SQ = 0.21443806949144176
B2 = -0.46791288034992956
KQ = 1.3508104959051634
# ~x reciprocal: rs ~= K_R*(z - A0)*bitcast(~bits(sn)), z = sn*bitcast(~bits(sn))
A0 = -8.5
K_R = -0.05545927
HALF_K = 0.5 * K_R


def _split_multi_waits(nc):
    """This container's walrus build rejects >1 sem-wait per instruction
    ("Too many sync wait commands"); hoist extras onto preceding NOPs."""
    for f in nc.m.functions:
        for blk in f.blocks:
            il = blk.instructions
            new = []
            for ins in il:
                si = ins.sync_info
                if si is not None and si.on_wait is not None and len(si.on_wait) > 1:
                    waits = list(si.on_wait)
                    for j, w in enumerate(waits[:-1]):
                        nop = mybir.InstNoOp(name=f"{ins.name}-ws{j}", engine=ins.engine)
                        nop.sync_info = mybir.SyncInfo(on_wait=[w], on_update=[])
                        new.append(nop)
                    ins.sync_info = mybir.SyncInfo(
                        on_wait=[waits[-1]], on_update=list(si.on_update or [])
                    )
                new.append(ins)
            il[:] = new


def _build_kernel():
    nc = bass.Bass()
    # extra const APs for activation biases (memsets + one barrier before
    # TileContext, same as the built-ins).
    for val in (B2, -0.5):
        t = nc.alloc_sbuf_tensor(f"const-f32-{val}", [128, 1], F32)
        nc.gpsimd.memset(t.ap(), val)
        nc.const_aps.aps[(F32, val)] = t.ap()
    nc.all_engine_barrier()

    x_in = nc.dram_tensor("mat_in", [N_C * 9], F32, kind="ExternalInput")
    y_out = nc.dram_tensor("twist_out", [N_C * 3], F32, kind="ExternalOutput")

    n = len(MS)
    offs = [0] + list(np.cumsum(MS)[:-1].astype(int))

    with TileContext(nc) as tc:
        with tc.tile_pool(name="xp", bufs=7) as xp, \
             tc.tile_pool(name="sp", bufs=4) as sp:

            X_, tr_, ax_, sg_, v_, qq_, nx_, tp_ = ({} for _ in range(8))

            def dma_in(i):
                m = MS[i]
                base = offs[i] * P * 9
                X_[i] = xp.tile([P, 9 * MAXM], F32, tag="X", name=f"X{i}")[:, : 9 * m]
                nc.sync.dma_start(
                    out=X_[i],
                    in_=x_in[base : base + P * 9 * m].rearrange("(p n) -> p n", p=P),
                )

            def gp_trace(i):
                m = MS[i]
                X = X_[i]
                # one scratch tile per chunk; slices: tr|ax|sg|v|qq|nx|tp
                s = sp.tile([P, 7 * MAXM], F32, tag="s", name=f"s{i}")
                tr_[i] = s[:, 0 * MAXM : 0 * MAXM + m]
                ax_[i] = s[:, 1 * MAXM : 1 * MAXM + m]
                sg_[i] = s[:, 2 * MAXM : 2 * MAXM + m]
                v_[i] = s[:, 3 * MAXM : 3 * MAXM + m]
                qq_[i] = s[:, 4 * MAXM : 4 * MAXM + m]
                nx_[i] = s[:, 5 * MAXM : 5 * MAXM + m]
                tp_[i] = s[:, 6 * MAXM : 6 * MAXM + m]
                tr = tr_[i]
                nc.gpsimd.tensor_add(
                    out=tr, in0=X[:, 6 * m : 7 * m], in1=X[:, 7 * m : 8 * m]
                )
                nc.gpsimd.tensor_add(out=tr, in0=tr, in1=X[:, 8 * m : 9 * m])

            def act_block(i):
                tr, ax, sg, v, qq = tr_[i], ax_[i], sg_[i], v_[i], qq_[i]
                nc.scalar.activation(ax, tr, ACT.Abs, scale=0.5, bias=-0.5)
                nc.scalar.activation(sg, tr, ACT.Sign, scale=0.5, bias=-0.5)
                nc.scalar.activation(v, tr, ACT.Square, scale=0.5, bias=-0.5)
                nc.scalar.activation(qq, ax, ACT.Square, scale=SQ, bias=B2)
                # in place: sn over v, S over ax (qq already consumed ax)
                nc.scalar.activation(v, v, ACT.Sqrt, scale=-1.0, bias=1.0)
                nc.scalar.activation(ax, ax, ACT.Sqrt, scale=-1.0, bias=1.0)

            def dve_a(i):
                m = MS[i]
                X = X_[i]
                sn, qq, nx = v_[i], qq_[i], nx_[i]
                # w = minu - subt, in place in X
                nc.vector.tensor_sub(
                    out=X[:, 0 : 3 * m], in0=X[:, 0 : 3 * m], in1=X[:, 3 * m : 6 * m]
                )
                # nx = bitcast(bits(sn) ^ -1)  (~= -C/sn)
                nc.vector.tensor_scalar(
                    out=nx.bitcast(I32), in0=sn.bitcast(I32),
                    scalar1=-1, scalar2=None, op0=ALU.bitwise_xor,
                )
                # u = (qq + KQ) * S = arccos(|x|)
                nc.vector.scalar_tensor_tensor(
                    out=qq, in0=qq, scalar=KQ, in1=ax_[i],
                    op0=ALU.add, op1=ALU.mult,
                )
                # tp = u - pi/2  (tensor_scalar runs 2x on fp32, STT only 1x)
                nc.vector.tensor_scalar(
                    out=tp_[i], in0=qq, scalar1=PI_2, scalar2=None,
                    op0=ALU.subtract,
                )

            def gp_zp(i):
                # z = sn*nx in [-4.5,-4] (over sn); p = tp*sign(x) (over tp)
                # Small multiplies on GpSimd; its consumers run a full
                # pipeline stage later so the handoff never stalls DVE.
                sn, nx, tp = v_[i], nx_[i], tp_[i]
                nc.gpsimd.tensor_mul(out=sn, in0=sn, in1=nx)
                nc.gpsimd.tensor_mul(out=tp, in0=tp, in1=sg_[i])

            def dve_b(i):
                m = MS[i]
                X = X_[i]
                sn, qq, nx, tp = v_[i], qq_[i], nx_[i], tp_[i]
                # A = (p + pi/2) * nx = theta * nx
                nc.vector.scalar_tensor_tensor(
                    out=qq, in0=tp, scalar=PI_2, in1=nx,
                    op0=ALU.add, op1=ALU.mult,
                )
                # P = (z - A0) * A = theta/(K_R*sin theta)
                nc.vector.scalar_tensor_tensor(
                    out=qq, in0=sn, scalar=A0, in1=qq,
                    op0=ALU.subtract, op1=ALU.mult,
                )
                # out_j = (HALF_K * P) * w_j, P broadcast over the 3 w-blocks
                bcast = qq.rearrange("p (o n) -> p o n", o=1).to_broadcast((P, 3, m))
                w3 = X[:, 0 : 3 * m].rearrange("p (k n) -> p k n", k=3)
                nc.vector.scalar_tensor_tensor(
                    out=w3, in0=bcast, scalar=HALF_K, in1=w3,
                    op0=ALU.mult, op1=ALU.mult,
                )

            def out_dma(i):
                m = MS[i]
                dst = y_out[offs[i] * P * 3 : (offs[i] + m) * P * 3].rearrange(
                    "(p n) -> p n", p=P
                )
                nc.scalar.dma_start(out=dst, in_=X_[i][:, 0 : 3 * m])

            def valid(j):
                return 0 <= j < n

            # software-pipelined emission; skew in iterations:
            # dma(i)@i, trace@i+1, acts@i+2, dveA+gp z/p@i+3, dveB@i+4,
            # out-dma@i+5 (trigger emitted at the top of the iteration so
            # the ACT-ring trigger never queues behind the ACT block).
            # gp z/p(i) consumers (dveB) run a full iteration later, so the
            # DVE->GpSimd->DVE handoff never stalls either in-order queue.
            for i in range(n + 5):
                if valid(i):
                    dma_in(i)
                if valid(i - 5):
                    out_dma(i - 5)
                if valid(i - 1):
                    gp_trace(i - 1)
                if valid(i - 3):
                    dve_a(i - 3)
                if valid(i - 3):
                    gp_zp(i - 3)
                if valid(i - 4):
                    dve_b(i - 4)
                if valid(i - 2):
                    act_block(i - 2)

    _split_multi_waits(nc)
    return nc


_NC_CACHE = []


def _host_pack(mat_batch: np.ndarray) -> np.ndarray:
    """[B,3,3] -> [NCORES, N_C*9] tile-major/component-major PERM layout."""
    flat = np.ascontiguousarray(mat_batch, dtype=np.float32).reshape(
        NCORES, N_C, 9
    )
    out = np.empty((NCORES, N_C * 9), np.float32)
    pos = 0
    for m, off in zip(MS, np.concatenate([[0], np.cumsum(MS)[:-1]])):
        off = int(off)
        chunk = flat[:, off * P : (off + m) * P, :].reshape(NCORES, P, m, 9)
        sz = P * m * 9
        out[:, pos : pos + sz] = (
            chunk.transpose(0, 1, 3, 2)[:, :, PERM, :].reshape(NCORES, sz)
        )
        pos += sz
    return out


def _host_unpack(res_list) -> np.ndarray:
    out = np.empty((B, 3), np.float32)
    o = out.reshape(NCORES, N_C, 3)
    for i, r in enumerate(res_list):
        y = r["twist_out"]
        pos = 0
        for m, off in zip(MS, np.concatenate([[0], np.cumsum(MS)[:-1]])):
            off = int(off)
            sz = P * m * 3
            blk = y[pos : pos + sz].reshape(P, 3, m)
            o[i, off * P : (off + m) * P, :] = blk.transpose(0, 2, 1).reshape(
                P * m, 3
            )
            pos += sz
    return out


def kernel(mat_batch: np.ndarray) -> np.ndarray:
    if not _NC_CACHE:
        _NC_CACHE.append(_build_kernel())
    nc = _NC_CACHE[0]

    packed = _host_pack(mat_batch)
    in_maps = [{"mat_in": packed[i]} for i in range(NCORES)]
    res = run_bass_kernel_spmd(nc, in_maps, core_ids=list(range(NCORES)))
    return _host_unpack(res.results)


# revision 23
# speedup vs baseline: 1.2479x; 1.2479x over previous
"""Mat2Twist Trainium2 kernel: batch of 3x3 rotation matrices -> twist vectors.

For each matrix R:  tr = trace(R); x = (tr-1)/2 = cos(theta)
  theta = arccos(x);  w = [R21-R12, R02-R20, R10-R01]  (|w| = 2 sin theta)
  out = theta/(2 sin theta) * w

arccos via the Abramowitz-Stegun-style split
  arccos(x) = pi/2 + sign(x)*(arccos(|x|) - pi/2),
  arccos(|x|) = sqrt(1-|x|) * q(|x|),  q deg-2 minimax (err ~1e-3)
so every activation (Abs, Sign, Square, Sqrt) lives in ONE ACT table set
("sqrt_and_others") -- a Square/Ln/Exp/Arctan mix would force ~2
ACT_TABLE_LOADs (1.3us each) per chunk and make the scalar engine the
pipeline straggler.  1/sin(theta) uses the bitwise-NOT trick: for s>0,
z = s*bitcast(~bits(s)) always lands in [-4.5,-4], and a single
Chebyshev-tuned factor K_R*(z-A0)*bitcast(~bits(s)) ~= 1/s to 1.7e-3
(output gate is 2e-2) -- two DVE ops, no iterative divide.  All
remaining constants fold into existing scale/scalar immediate slots.

Per chunk (m matrices per partition), tile X = [minu(3m)|subt(3m)|R00|R11|R22]:
  gp:  tr = R00+R11+R22                       [GpSimd TT x2 -- anything
       more on GpSimd inflates every engine via SBUF-port contention]
  act: ax=Abs(.5tr-.5) sg=Sign(.5tr-.5) v=Square(.5tr-.5)   (x=(tr-1)/2)
       qq=Square(SQ*ax+B2)  sn=Sqrt(1-v)  S=Sqrt(1-ax)  [6 ACT, one set]
  dve: w  = minu-subt (in place)              [fp32 TT/STT are 1x-capped;
       nx = bitcast(bits(sn) ^ -1)  [int ts]  this chain is the minimum
       z  = sn*nx                   [TT]      op count]
       u  = (qq+KQ)*S       = arccos(|x|)
       p  = (u-pi/2)*sg     = theta - pi/2
       A  = (p+pi/2)*nx     = theta*nx
       P  = (z-A0)*A        = theta/(K_R*sin theta)
       out_j = (HALF_K*P)*w_j  as one STT over [P,3,m], P broadcast
  out-DMA on the ACT ring, inputs on the SP ring.

The small intermediates live as slices of ONE scratch tile per chunk.
Chunks are large (m=640) to amortize the per-op fixed costs (DVE +151cy,
ACT +352cy) and to cut total instruction count: teardown ends with a
per-instruction event-semaphore lockstep walk across all six engines, so
fewer instructions also means a shorter tail.  Two m=128 tail chunks
keep the post-last-input drain latency low.

Software-pipelined emission, skew: dma(i)@i -> trace@i+1 -> acts@i+2 ->
dve@i+3 -> out-dma trigger@i+4 (trigger emitted at the top of its
iteration so it never queues behind that iteration's ACT block).
"""

import numpy as np

import concourse.bass as bass
import concourse.mybir as mybir
from concourse.tile import TileContext
from concourse.bass_utils import run_bass_kernel_spmd

B = 4194304
NCORES = 8
P = 128
N_C = B // NCORES        # 524288 matrices per core
MPP = N_C // P           # 4096 matrices per partition
MS = [640] * 6 + [128] * 2   # per-chunk matrices per partition
assert sum(MS) == MPP

# component order in DRAM (flat 3x3 index): minuends, subtrahends, diagonal
PERM = [7, 2, 3, 5, 6, 1, 0, 4, 8]

F32 = mybir.dt.float32
I32 = mybir.dt.int32
ACT = mybir.ActivationFunctionType
ALU = mybir.AluOpType
PI_2 = float(np.pi / 2.0)
MAXM = max(MS)

# deg-2 minimax fit of q(t) = arccos(t)/sqrt(1-t) on [0, cos(0.1)],
# written as q = (SQ*t + B2)^2 + KQ so one ACT Square evaluates it.
SQ = 0.21443806949144176
B2 = -0.46791288034992956
KQ = 1.3508104959051634
# ~x reciprocal: 1/sn ~= K_R*(z - A0)*bitcast(~bits(sn)), z = sn*bitcast(~bits(sn))
A0 = -8.5
K_R = -0.05545927
HALF_K = 0.5 * K_R


def _split_multi_waits(nc):
    """This container's walrus build rejects >1 sem-wait per instruction
    ("Too many sync wait commands"); hoist extras onto preceding NOPs."""
    for f in nc.m.functions:
        for blk in f.blocks:
            il = blk.instructions
            new = []
            for ins in il:
                si = ins.sync_info
                if si is not None and si.on_wait is not None and len(si.on_wait) > 1:
                    waits = list(si.on_wait)
                    for j, w in enumerate(waits[:-1]):
                        nop = mybir.InstNoOp(name=f"{ins.name}-ws{j}", engine=ins.engine)
                        nop.sync_info = mybir.SyncInfo(on_wait=[w], on_update=[])
                        new.append(nop)
                    ins.sync_info = mybir.SyncInfo(
                        on_wait=[waits[-1]], on_update=list(si.on_update or [])
                    )
                new.append(ins)
            il[:] = new


def _build_kernel():
    nc = bass.Bass()
    # extra const APs for activation biases (memsets + one barrier before
    # TileContext, same as the built-ins).
    for val in (B2, -0.5):
        t = nc.alloc_sbuf_tensor(f"const-f32-{val}", [128, 1], F32)
        nc.gpsimd.memset(t.ap(), val)
        nc.const_aps.aps[(F32, val)] = t.ap()
    nc.all_engine_barrier()

    x_in = nc.dram_tensor("mat_in", [N_C * 9], F32, kind="ExternalInput")
    y_out = nc.dram_tensor("twist_out", [N_C * 3], F32, kind="ExternalOutput")

    n = len(MS)
    offs = [0] + list(np.cumsum(MS)[:-1].astype(int))

    with TileContext(nc) as tc:
        with tc.tile_pool(name="xp", bufs=6) as xp, \
             tc.tile_pool(name="sp", bufs=3) as sp:

            X_, tr_, ax_, sg_, v_, qq_, nx_ = ({} for _ in range(7))

            def dma_in(i):
                m = MS[i]
                base = offs[i] * P * 9
                X_[i] = xp.tile([P, 9 * MAXM], F32, tag="X", name=f"X{i}")[:, : 9 * m]
                nc.sync.dma_start(
                    out=X_[i],
                    in_=x_in[base : base + P * 9 * m].rearrange("(p n) -> p n", p=P),
                )

            def gp_trace(i):
                m = MS[i]
                X = X_[i]
                # one scratch tile per chunk; slices: tr|ax|sg|v|qq|nx
                s = sp.tile([P, 6 * MAXM], F32, tag="s", name=f"s{i}")
                tr_[i] = s[:, 0 * MAXM : 0 * MAXM + m]
                ax_[i] = s[:, 1 * MAXM : 1 * MAXM + m]
                sg_[i] = s[:, 2 * MAXM : 2 * MAXM + m]
                v_[i] = s[:, 3 * MAXM : 3 * MAXM + m]
                qq_[i] = s[:, 4 * MAXM : 4 * MAXM + m]
                nx_[i] = s[:, 5 * MAXM : 5 * MAXM + m]
                tr = tr_[i]
                nc.gpsimd.tensor_add(
                    out=tr, in0=X[:, 6 * m : 7 * m], in1=X[:, 7 * m : 8 * m]
                )
                nc.gpsimd.tensor_add(out=tr, in0=tr, in1=X[:, 8 * m : 9 * m])

            def act_block(i):
                tr, ax, sg, v, qq = tr_[i], ax_[i], sg_[i], v_[i], qq_[i]
                nc.scalar.activation(ax, tr, ACT.Abs, scale=0.5, bias=-0.5)
                nc.scalar.activation(sg, tr, ACT.Sign, scale=0.5, bias=-0.5)
                nc.scalar.activation(v, tr, ACT.Square, scale=0.5, bias=-0.5)
                nc.scalar.activation(qq, ax, ACT.Square, scale=SQ, bias=B2)
                # in place: sn over v, S over ax (qq already consumed ax)
                nc.scalar.activation(v, v, ACT.Sqrt, scale=-1.0, bias=1.0)
                nc.scalar.activation(ax, ax, ACT.Sqrt, scale=-1.0, bias=1.0)

            def dve_block(i):
                m = MS[i]
                X = X_[i]
                sn, qq, nx = v_[i], qq_[i], nx_[i]
                # w = minu - subt, in place in X
                nc.vector.tensor_sub(
                    out=X[:, 0 : 3 * m], in0=X[:, 0 : 3 * m], in1=X[:, 3 * m : 6 * m]
                )
                # nx = bitcast(bits(sn) ^ -1) (~= -C/sn); z = sn*nx in [-4.5,-4]
                nc.vector.tensor_scalar(
                    out=nx.bitcast(I32), in0=sn.bitcast(I32),
                    scalar1=-1, scalar2=None, op0=ALU.bitwise_xor,
                )
                nc.vector.tensor_mul(out=sn, in0=sn, in1=nx)  # z over sn
                # u = (qq + KQ) * S = arccos(|x|)
                nc.vector.scalar_tensor_tensor(
                    out=qq, in0=qq, scalar=KQ, in1=ax_[i],
                    op0=ALU.add, op1=ALU.mult,
                )
                # p = (u - pi/2) * sign(x) = theta - pi/2
                nc.vector.scalar_tensor_tensor(
                    out=qq, in0=qq, scalar=PI_2, in1=sg_[i],
                    op0=ALU.subtract, op1=ALU.mult,
                )
                # A = (p + pi/2) * nx = theta * nx
                nc.vector.scalar_tensor_tensor(
                    out=qq, in0=qq, scalar=PI_2, in1=nx,
                    op0=ALU.add, op1=ALU.mult,
                )
                # P = (z - A0) * A = theta/(K_R*sin theta)
                nc.vector.scalar_tensor_tensor(
                    out=qq, in0=sn, scalar=A0, in1=qq,
                    op0=ALU.subtract, op1=ALU.mult,
                )
                # out_j = (HALF_K * P) * w_j, P broadcast over the 3 w-blocks
                bcast = qq.rearrange("p (o n) -> p o n", o=1).to_broadcast((P, 3, m))
                w3 = X[:, 0 : 3 * m].rearrange("p (k n) -> p k n", k=3)
                nc.vector.scalar_tensor_tensor(
                    out=w3, in0=bcast, scalar=HALF_K, in1=w3,
                    op0=ALU.mult, op1=ALU.mult,
                )

            def out_dma(i):
                m = MS[i]
                dst = y_out[offs[i] * P * 3 : (offs[i] + m) * P * 3].rearrange(
                    "(p n) -> p n", p=P
                )
                nc.scalar.dma_start(out=dst, in_=X_[i][:, 0 : 3 * m])

            def valid(j):
                return 0 <= j < n

            # software-pipelined emission; skew in iterations:
            # dma(i)@i, trace@i+1, acts@i+2, dve@i+3, out-dma@i+4 (trigger
            # emitted at the top of the iteration so the ACT-ring trigger
            # never queues behind that iteration's ACT block)
            for i in range(n + 4):
                if valid(i):
                    dma_in(i)
                if valid(i - 4):
                    out_dma(i - 4)
                if valid(i - 1):
                    gp_trace(i - 1)
                if valid(i - 3):
                    dve_block(i - 3)
                if valid(i - 2):
                    act_block(i - 2)

    _split_multi_waits(nc)
    return nc


_NC_CACHE = []


def _host_pack(mat_batch: np.ndarray) -> np.ndarray:
    """[B,3,3] -> [NCORES, N_C*9] tile-major/component-major PERM layout."""
    flat = np.ascontiguousarray(mat_batch, dtype=np.float32).reshape(
        NCORES, N_C, 9
    )
    out = np.empty((NCORES, N_C * 9), np.float32)
    pos = 0
    for m, off in zip(MS, np.concatenate([[0], np.cumsum(MS)[:-1]])):
        off = int(off)
        chunk = flat[:, off * P : (off + m) * P, :].reshape(NCORES, P, m, 9)
        sz = P * m * 9
        out[:, pos : pos + sz] = (
            chunk.transpose(0, 1, 3, 2)[:, :, PERM, :].reshape(NCORES, sz)
        )
        pos += sz
    return out


def _host_unpack(res_list) -> np.ndarray:
    out = np.empty((B, 3), np.float32)
    o = out.reshape(NCORES, N_C, 3)
    for i, r in enumerate(res_list):
        y = r["twist_out"]
        pos = 0
        for m, off in zip(MS, np.concatenate([[0], np.cumsum(MS)[:-1]])):
            off = int(off)
            sz = P * m * 3
            blk = y[pos : pos + sz].reshape(P, 3, m)
            o[i, off * P : (off + m) * P, :] = blk.transpose(0, 2, 1).reshape(
                P * m, 3
            )
            pos += sz
    return out


def kernel(mat_batch: np.ndarray) -> np.ndarray:
    if not _NC_CACHE:
        _NC_CACHE.append(_build_kernel())
    nc = _NC_CACHE[0]

    packed = _host_pack(mat_batch)
    in_maps = [{"mat_in": packed[i]} for i in range(NCORES)]
    res = run_bass_kernel_spmd(nc, in_maps, core_ids=list(range(NCORES)))
    return _host_unpack(res.results)


# revision 25
# speedup vs baseline: 1.3860x; 1.1107x over previous
"""Mat2Twist Trainium2 kernel: batch of 3x3 rotation matrices -> twist vectors.

For each matrix R:  tr = trace(R); x = (tr-1)/2 = cos(theta)
  theta = arccos(x) = pi/2 - arctan(x / sqrt(1 - x^2))
  w = [R21-R12, R02-R20, R10-R01]   (unnormalized axis, |w| = 2 sin theta)
  out = theta/(2 sin theta) * w

Per chunk (m matrices per partition), tile X = [minu(3m)|subt(3m)|R00|R11|R22]:
  tr   = R00 + R11 + R22                           [GpSimd TT x2]
  v    = Square(0.5*tr - 0.5)     = x^2            [ACT set6]
  lg   = Ln(1 - v)                                 [ACT set6]
  r    = Exp(-0.5*lg)             = 1/sin theta    [ACT set6]
  w    = X[0:3m] - X[3m:6m]       in place         [DVE TT]
  xr   = (tr - 1) * r             = 2 cot theta    [DVE STT]
  t    = Arctan(0.5*xr)           = pi/2 - theta   [ACT set2, in place]
  msc2 = (t - pi/2) * r           = -theta/sin th  [DVE STT]
  out_k = (-0.5*msc2) * w_k                        [DVE STT over [P,3,m]]

The ACT engine holds ONE table set at a time; Square/Ln/Exp live in
"natural_log_exp_and_others" but Arctan does not, so a per-chunk op
order pays 2 x 1.3us ACT_TABLE_LOAD per chunk (~23us of scalar time).
Instead the Arctan ops are BATCHED per 3-chunk group: the scalar queue
sees runs [Sq,Ln,Exp]x3 then [Arctan]x3 -> 2 loads per 3 chunks.  The
msc2/out-mul DVE ops and output triggers batch along with them.

Keeping the small-DVE-op count at 2 per chunk matters more than table
loads: fp32 DVE tensor ops are 1x-capped (~(N+151)cy/0.96GHz) and every
engine's ops inflate up to 2-4x under concurrent DMA+GpSimd SBUF-port
traffic, so alternative arccos formulations with more DVE/GpSimd ops
lose more to contention than they save on the scalar engine.

All engines are in-order queues; emission is software-pipelined:
dma(i)@i -> trace@i+1 -> sq/ln/exp@i+2 -> sub/xr@i+3, and at each
3-chunk group boundary the [arctan x3][msc2+muls x3] batches, with the
out-DMA triggers (ACT ring) emitted at the top of the following
iteration.  Inputs ride the SP ring, outputs the ACT ring in parallel.
Tail chunks are small to shorten the pipeline drain.
"""

import numpy as np

import concourse.bass as bass
import concourse.mybir as mybir
from concourse.tile import TileContext
from concourse.bass_utils import run_bass_kernel_spmd

B = 4194304
NCORES = 8
P = 128
N_C = B // NCORES        # 524288 matrices per core
MPP = N_C // P           # 4096 matrices per partition
MS = [512] * 7 + [256] + [128] * 2   # per-chunk matrices per partition
assert sum(MS) == MPP
GROUP = 3

# component order in DRAM (flat 3x3 index): minuends, subtrahends, diagonal
PERM = [7, 2, 3, 5, 6, 1, 0, 4, 8]

F32 = mybir.dt.float32
ACT = mybir.ActivationFunctionType
ALU = mybir.AluOpType
PI_2 = float(np.pi / 2.0)
MAXM = max(MS)


def _split_multi_waits(nc):
    """This container's walrus build rejects >1 sem-wait per instruction
    ("Too many sync wait commands"); hoist extras onto preceding NOPs."""
    for f in nc.m.functions:
        for blk in f.blocks:
            il = blk.instructions
            new = []
            for ins in il:
                si = ins.sync_info
                if si is not None and si.on_wait is not None and len(si.on_wait) > 1:
                    waits = list(si.on_wait)
                    for j, w in enumerate(waits[:-1]):
                        nop = mybir.InstNoOp(name=f"{ins.name}-ws{j}", engine=ins.engine)
                        nop.sync_info = mybir.SyncInfo(on_wait=[w], on_update=[])
                        new.append(nop)
                    ins.sync_info = mybir.SyncInfo(
                        on_wait=[waits[-1]], on_update=list(si.on_update or [])
                    )
                new.append(ins)
            il[:] = new


def _build_kernel():
    nc = bass.Bass()
    # extra const AP for the -0.5 activation bias (memset + barrier before
    # TileContext, same as the built-ins).
    t = nc.alloc_sbuf_tensor("const-f32--0.5", [128, 1], F32)
    nc.gpsimd.memset(t.ap(), -0.5)
    nc.const_aps.aps[(F32, -0.5)] = t.ap()
    nc.all_engine_barrier()

    x_in = nc.dram_tensor("mat_in", [N_C * 9], F32, kind="ExternalInput")
    y_out = nc.dram_tensor("twist_out", [N_C * 3], F32, kind="ExternalOutput")

    n = len(MS)
    offs = [0] + list(np.cumsum(MS)[:-1].astype(int))

    with TileContext(nc) as tc:
        with tc.tile_pool(name="xp", bufs=8) as xp, \
             tc.tile_pool(name="tlong", bufs=6) as tlong, \
             tc.tile_pool(name="tshort", bufs=3) as tshort:

            X_, tr_, r_, xr_ = {}, {}, {}, {}

            def dma_in(i):
                m = MS[i]
                base = offs[i] * P * 9
                X_[i] = xp.tile([P, 9 * MAXM], F32, tag="X", name=f"X{i}")[:, : 9 * m]
                nc.sync.dma_start(
                    out=X_[i],
                    in_=x_in[base : base + P * 9 * m].rearrange("(p n) -> p n", p=P),
                )

            def gp_trace(i):
                m = MS[i]
                X = X_[i]
                tr = tlong.tile([P, MAXM], F32, tag="tr", name=f"tr{i}", bufs=4)[:, :m]
                nc.gpsimd.tensor_add(
                    out=tr, in0=X[:, 6 * m : 7 * m], in1=X[:, 7 * m : 8 * m]
                )
                nc.gpsimd.tensor_add(out=tr, in0=tr, in1=X[:, 8 * m : 9 * m])
                tr_[i] = tr

            def act_lnexp(i):
                m = MS[i]
                tr = tr_[i]
                v = tshort.tile([P, MAXM], F32, tag="v", name=f"v{i}")[:, :m]
                nc.scalar.activation(v, tr, ACT.Square, scale=0.5, bias=-0.5)
                lg = tshort.tile([P, MAXM], F32, tag="lg", name=f"lg{i}")[:, :m]
                nc.scalar.activation(lg, v, ACT.Ln, bias=1.0, scale=-1.0)
                r = tlong.tile([P, MAXM], F32, tag="r", name=f"r{i}", bufs=6)[:, :m]
                nc.scalar.activation(r, lg, ACT.Exp, scale=-0.5)
                r_[i] = r

            def dve_subxr(i):
                m = MS[i]
                X = X_[i]
                nc.vector.tensor_sub(
                    out=X[:, 0 : 3 * m], in0=X[:, 0 : 3 * m], in1=X[:, 3 * m : 6 * m]
                )
                xr = tlong.tile([P, MAXM], F32, tag="xr", name=f"xr{i}", bufs=5)[:, :m]
                nc.vector.scalar_tensor_tensor(
                    out=xr, in0=tr_[i], scalar=1.0, in1=r_[i],
                    op0=ALU.subtract, op1=ALU.mult,
                )
                xr_[i] = xr

            def act_arctan(i):
                nc.scalar.activation(xr_[i], xr_[i], ACT.Arctan, scale=0.5)

            def dve_out(i):
                m = MS[i]
                X = X_[i]
                msc2 = tshort.tile([P, MAXM], F32, tag="msc2", name=f"msc2{i}")[:, :m]
                nc.vector.scalar_tensor_tensor(
                    out=msc2, in0=xr_[i], scalar=PI_2, in1=r_[i],
                    op0=ALU.subtract, op1=ALU.mult,
                )
                # one STT over all 3 w-blocks with msc2 broadcast (stride-0 dim)
                bcast = msc2.rearrange("p (o n) -> p o n", o=1).to_broadcast((P, 3, m))
                w3 = X[:, 0 : 3 * m].rearrange("p (k n) -> p k n", k=3)
                nc.vector.scalar_tensor_tensor(
                    out=w3, in0=bcast, scalar=-0.5, in1=w3,
                    op0=ALU.mult, op1=ALU.mult,
                )

            def out_dma(i):
                m = MS[i]
                dst = y_out[offs[i] * P * 3 : (offs[i] + m) * P * 3].rearrange(
                    "(p n) -> p n", p=P
                )
                nc.scalar.dma_start(out=dst, in_=X_[i][:, 0 : 3 * m])

            def valid(j):
                return 0 <= j < n

            # Software-pipelined emission.  Per-chunk stages:
            #   dma(i)@i, trace(i)@i+1, sq/ln/exp(i)@i+2, sub/xr(i)@i+3.
            # When chunk j = i-3 completes a 3-chunk group (or is the last
            # chunk), emit [arctan]xG then [msc2+muls]xG for the group --
            # the scalar queue sees same-table-set runs (2 ACT_TABLE_LOADs
            # per group instead of 2 per chunk).  Output triggers for the
            # group are emitted at the top of subsequent iterations.
            pend_out = []   # chunks whose out-DMA trigger is not yet emitted
            for i in range(n + 6):
                if valid(i):
                    dma_in(i)
                if pend_out:
                    out_dma(pend_out.pop(0))
                if valid(i - 1):
                    gp_trace(i - 1)
                if valid(i - 2):
                    act_lnexp(i - 2)
                if valid(i - 3):
                    dve_subxr(i - 3)
                j = i - 3
                if valid(j) and (j % GROUP == GROUP - 1 or j == n - 1):
                    g0 = j - (j % GROUP)
                    for c in range(g0, j + 1):
                        act_arctan(c)
                    for c in range(g0, j + 1):
                        dve_out(c)
                    pend_out.extend(range(g0, j + 1))

    _split_multi_waits(nc)
    return nc


_NC_CACHE = []


def _host_pack(mat_batch: np.ndarray) -> np.ndarray:
    """[B,3,3] -> [NCORES, N_C*9] tile-major/component-major PERM layout."""
    flat = np.ascontiguousarray(mat_batch, dtype=np.float32).reshape(
        NCORES, N_C, 9
    )
    out = np.empty((NCORES, N_C * 9), np.float32)
    pos = 0
    for m, off in zip(MS, np.concatenate([[0], np.cumsum(MS)[:-1]])):
        off = int(off)
        chunk = flat[:, off * P : (off + m) * P, :].reshape(NCORES, P, m, 9)
        sz = P * m * 9
        out[:, pos : pos + sz] = (
            chunk.transpose(0, 1, 3, 2)[:, :, PERM, :].reshape(NCORES, sz)
        )
        pos += sz
    return out


def _host_unpack(res_list) -> np.ndarray:
    out = np.empty((B, 3), np.float32)
    o = out.reshape(NCORES, N_C, 3)
    for i, r in enumerate(res_list):
        y = r["twist_out"]
        pos = 0
        for m, off in zip(MS, np.concatenate([[0], np.cumsum(MS)[:-1]])):
            off = int(off)
            sz = P * m * 3
            blk = y[pos : pos + sz].reshape(P, 3, m)
            o[i, off * P : (off + m) * P, :] = blk.transpose(0, 2, 1).reshape(
                P * m, 3
            )
            pos += sz
    return out


def kernel(mat_batch: np.ndarray) -> np.ndarray:
    if not _NC_CACHE:
        _NC_CACHE.append(_build_kernel())
    nc = _NC_CACHE[0]

    packed = _host_pack(mat_batch)
    in_maps = [{"mat_in": packed[i]} for i in range(NCORES)]
    res = run_bass_kernel_spmd(nc, in_maps, core_ids=list(range(NCORES)))
    return _host_unpack(res.results)
